# revision 42
# baseline (speedup 1.0000x reference)
"""CrossCompress unit kernel for Trainium2, 8-core data parallel.

Reference computation (per batch row b, D=128):
    item_out[b]   = v[b] * (e[b]@w_vv) + e[b] * (v[b]@w_ev) + bias_v
    entity_out[b] = v[b] * (e[b]@w_ve) + e[b] * (v[b]@w_ee) + bias_e

Strategy: pure data parallel over B=16384 rows -> 2048 rows/core.
Each core works in a transposed layout [D=128 partitions, batch free]:
the four per-row dot products become PE matmuls whose stationary operand
is the (D,1) weight replicated across 128 columns -- one matmul both
computes the dots AND broadcasts the result down all partitions.

All I/O and SBUF data is fp16 (PE fp16 1 cycle/row; DMA bytes halve).
PSUM accumulation stays fp32. Global rel error ~1e-3 (gate 2e-2).

Supertiles NS=(384,512,512,384,256) -- a mid-size first tile gives the
DVE enough head work to cover the PE's cold-clock matmul phase, and a
small last tile keeps the drain short. Per supertile:
  PE   : 4 dot+broadcast matmuls into ONE psum tile whose four slots sit
         at 512-col (2KB-bank) strides -- a matmul's psum output must
         not cross a bank boundary, so slots are bank-padded for N<512
  DVE  : ONE quad-packed product t = [v,v,e,e] (*) [a,b,c,d] (stride-0
         broadcast read of v/e; DVE is the only PSUM-capable tensor-
         tensor engine, so its ~9.4us over the 8192 product-columns is
         the structural floor that paces the body)
  Pool : per-half sums item = a*v + c*e, ent = b*v + d*e (st0..st3),
         written straight into the output tile when biases are zero
         (the reference setup); the Act bias stage only exists on the
         has_bias build
  last : one pair-packed 2x-mode tensor add on DVE; its two half DMAs
         issue from the idle GpSimd and Act queues in parallel to cut
         the drain tail
Each half rides its own SP-issued DMA so outputs stream while the next
half computes. The consts ride the FIRST supertile's input DMA (one
issue+transfer+semaphore chain on the critical path instead of two).
ONLY that first DMA is hoisted in front of the framework preamble
barriers: parked at the head of the SP queue it fires the moment the
runtime's ~6.7us DMA-enable gate opens, so its transfer overlaps the
preamble -- while hoisting MORE DMAs delays SP's arrival at the
preamble ring barrier and pushes every engine's start out (measured
+0.4us at k=5 vs k=1).

Walrus CoreV3 codegen accepts only ONE embedded sync wait per
instruction; a post-pass splits any multi-wait instruction into
single-wait NoOps.
"""
import sys
sys.path.insert(0, '/opt/trn_rl_repo')
import numpy as np
import bass_rust
import concourse.bass as bass
import concourse.tile as tile
from concourse import mybir
from concourse.bass_utils import run_bass_kernel_spmd

B, D = 16384, 128
NCORES = 8
RPC = B // NCORES          # rows per core = 2048
NS = (384, 512, 512, 384, 256)  # supertile batch-column counts
WARMUP_MM = 0              # PE p-state warmup matmuls (prefetch makes them net-negative)
assert sum(NS) == RPC
CW = 4 * D + 2             # const block: 4 replicated weights + 2 biases

F32 = mybir.dt.float32
F16 = mybir.dt.float16


def _build(has_bias):
    nc = bass.Bass("TRN2", target_bir_lowering=False, debug=False,
                   num_devices=NCORES)
    # flat input per core: [D, CW + 2*RPC]: [consts | st0 v|e | st1 v|e ...]
    xin = nc.dram_tensor("xin", [D, CW + 2 * RPC], F16,
                         kind="ExternalInput").ap()
    out = nc.dram_tensor("out", [D, 2 * RPC], F16, kind="ExternalOutput").ap()

    nst = len(NS)
    with tile.TileContext(nc) as tc:
        with tc.tile_pool(name="c0", bufs=1) as c0_pool, \
             tc.tile_pool(name="io", bufs=6) as io_pool, \
             tc.tile_pool(name="ob", bufs=5) as ob_pool, \
             tc.tile_pool(name="tmp", bufs=5) as tmp_pool, \
             tc.tile_pool(name="ps", bufs=2, space="PSUM") as ps_pool:

            # consts + the first supertile ride ONE DMA (they are
            # contiguous in xin) so the first matmul waits on a single
            # issue+transfer+semaphore chain instead of two
            N0 = NS[0]
            c0_sb = c0_pool.tile([D, CW + 2 * N0], F16)
            nc.sync.dma_start(out=c0_sb[:], in_=xin[:, 0:CW + 2 * N0])
            # PE p-state warmup: matmuls on a never-DMA'd scratch tile (no
            # producer -> no waits) keep the PE continuously busy through
            # the preamble+DMA window so real matmuls run at full clock.
            if WARMUP_MM:
                wup = c0_pool.tile([D, 2 * D], F16, tag="wup")
                nc.gpsimd.memset(wup[:], 0.0)
            w_sb = c0_sb[:, 0:4 * D]
            bv_sb = c0_sb[:, 4 * D:4 * D + 1]
            be_sb = c0_sb[:, 4 * D + 1:CW]

            if WARMUP_MM:
                wps = ps_pool.tile([D, 2, 2, 512], F32, tag="sAB",
                                   name="warmup_ps")
                for k in range(WARMUP_MM):
                    nc.tensor.matmul(wps[:, k % 2, k // 2 % 2, 0:2 * D],
                                     wup[:, 0:D], wup[:], start=True,
                                     stop=True)

            in_off = CW + 2 * N0
            out_off = 0
            for st, N in enumerate(NS):
                if st == 0:
                    ve_sb = c0_sb[:, CW:CW + 2 * N0]
                else:
                    ve_t = io_pool.tile([D, 2 * N], F16, tag="ve",
                                        name=f"ve_{st}")
                    nc.sync.dma_start(out=ve_t[:],
                                      in_=xin[:, in_off:in_off + 2 * N])
                    in_off += 2 * N
                    ve_sb = ve_t[:]
                v_sb = ve_sb[:, 0:N]
                e_sb = ve_sb[:, N:2 * N]

                # dot+broadcast matmuls, one 4-slot psum tile:
                # sAB = [e@w_vv | e@w_ve | v@w_ev | v@w_ee] = [a|b|c|d]
                # each dot-product slot gets a FULL 2KB psum bank (a
                # matmul's psum output must not cross a bank boundary);
                # only the first N columns of each bank are written/read
                sAB = ps_pool.tile([D, 2, 2, 512], F32, tag="sAB",
                                   name=f"sAB_{st}")
                nc.tensor.matmul(sAB[:, 0, 0, 0:N], w_sb[:, 0 * D:1 * D],
                                 e_sb, start=True, stop=True)
                nc.tensor.matmul(sAB[:, 0, 1, 0:N], w_sb[:, 2 * D:3 * D],
                                 e_sb, start=True, stop=True)
                nc.tensor.matmul(sAB[:, 1, 0, 0:N], w_sb[:, 1 * D:2 * D],
                                 v_sb, start=True, stop=True)
                nc.tensor.matmul(sAB[:, 1, 1, 0:N], w_sb[:, 3 * D:4 * D],
                                 v_sb, start=True, stop=True)

                # ONE quad-packed product on DVE: t = [v,v,e,e] (*) sAB
                # (in0 reads ve_sb as [D, {v,e}, x2, N] with a stride-0 dim)
                t = tmp_pool.tile([D, 2, 2, N], F16, tag="t", name=f"t_{st}")
                in0 = ve_sb.rearrange("p (b n) -> p b n", b=2).unsqueeze(
                    2).broadcast_to([D, 2, 2, N])
                nc.vector.tensor_mul(t[:], in0, sAB[:, :, :, 0:N])
                t_v = t[:, 0]     # [a*v | b*v]
                t_e = t[:, 1]     # [c*e | d*e]

                o_sb = ob_pool.tile([D, 2, N], F16, tag="o", name=f"o_{st}")
                if st < nst - 1:
                    # half-granular add(+bias)->DMA so the tail of each tile
                    # streams out while the other half is still in flight.
                    # With all-zero biases (the reference setup) the Act
                    # bias stage is elided and the GpSimd sum writes the
                    # output tile directly.
                    if has_bias:
                        ts = tmp_pool.tile([D, 2, N], F16, tag="ts",
                                           name=f"ts_{st}")
                    for h, bias in ((0, bv_sb), (1, be_sb)):
                        if has_bias:
                            nc.gpsimd.tensor_add(ts[:, h], t[:, 0, h],
                                                 t[:, 1, h])
                            nc.scalar.activation(
                                o_sb[:, h], ts[:, h],
                                mybir.ActivationFunctionType.Identity,
                                bias=bias, scale=1.0)
                        else:
                            nc.gpsimd.tensor_add(o_sb[:, h], t[:, 0, h],
                                                 t[:, 1, h])
                        nc.sync.dma_start(
                            out=out[:, out_off + h * N:out_off + (h + 1) * N],
                            in_=o_sb[:, h])
                else:
                    # last tile on DVE: fused (t_v+bias)+t_e, or a plain
                    # 2x-mode tensor add when biases are zero; half DMAs
                    # issue from the idle GpSimd/Act queues in parallel
                    if has_bias:
                        for h, bias in ((0, bv_sb), (1, be_sb)):
                            nc.vector.scalar_tensor_tensor(
                                o_sb[:, h], t[:, 0, h], bias, t[:, 1, h],
                                op0=mybir.AluOpType.add,
                                op1=mybir.AluOpType.add)
                    else:
                        # one pair-packed 2x-mode add covers both halves
                        nc.vector.tensor_add(o_sb[:], t[:, 0], t[:, 1])
                    for h, eng in ((0, nc.gpsimd), (1, nc.scalar)):
                        eng.dma_start(
                            out=out[:, out_off + h * N:out_off + (h + 1) * N],
                            in_=o_sb[:, h])
                out_off += 2 * N
    _split_multiwaits(nc)
    _hoist_first_dmas(nc, k=1)
    return nc


def _hoist_first_dmas(nc, k):
    """Move the first k wait-free SP DMA instructions to the very front of
    the first block, ahead of the framework preamble barriers, so their
    ~2.3us issue+transfer+semaphore latency hides under the ~7us engine
    spin-up window. Their semaphore updates are unchanged -- consumers
    still wait on the same counts."""
    blocks = nc.m.functions[0].blocks
    moved = []
    for b in blocks:
        insts = b.instructions
        keep = []
        for inst in insts:
            if (len(moved) < k and 'DMA' in str(inst.opcode)
                    and inst.engine == mybir.EngineType.SP
                    and (inst.sync_info is None
                         or not inst.sync_info.on_wait)):
                moved.append(inst)
            else:
                keep.append(inst)
        insts[:] = keep
        if len(moved) >= k:
            break
    blocks[0].instructions[:0] = moved
    return len(moved)


def _split_multiwaits(nc):
    """Split instructions carrying >1 sync wait into single-wait NoOps
    inserted just before them on the same engine queue."""
    n = 0
    for b in nc.m.functions[0].blocks:
        insts = b.instructions
        new = []
        for inst in insts:
            si = inst.sync_info
            if si is not None and si.on_wait and len(si.on_wait) > 1:
                waits = list(si.on_wait)
                for k, w in enumerate(waits[:-1]):
                    nop = mybir.InstNoOp(name=f"{inst.name}-sw{k}",
                                         ins=[], outs=[])
                    nop.engine = inst.engine
                    nop.sync_info = bass_rust.SyncInfo(on_wait=[w],
                                                       on_update=[])
                    nc.register_instruction(nop)
                    new.append(nop)
                    n += 1
                si.on_wait = [waits[-1]]
            new.append(inst)
        insts[:] = new
    return n


_NC = {}


def _get_nc(has_bias):
    if has_bias not in _NC:
        _NC[has_bias] = _build(has_bias)
    return _NC[has_bias]


def _make_in_maps(v, e, w_vv, w_ve, w_ev, w_ee, bias_v, bias_e):
    cst = np.empty((D, CW), np.float16)
    cst[:, 0 * D:1 * D] = np.repeat(w_vv.reshape(D, 1), D, axis=1)
    cst[:, 1 * D:2 * D] = np.repeat(w_ev.reshape(D, 1), D, axis=1)
    cst[:, 2 * D:3 * D] = np.repeat(w_ve.reshape(D, 1), D, axis=1)
    cst[:, 3 * D:4 * D] = np.repeat(w_ee.reshape(D, 1), D, axis=1)
    cst[:, 4 * D] = bias_v.reshape(D)
    cst[:, 4 * D + 1] = bias_e.reshape(D)

    vT = np.ascontiguousarray(v.T).astype(np.float16)   # [D, B]
    eT = np.ascontiguousarray(e.T).astype(np.float16)
    in_maps = []
    for c in range(NCORES):
        xin = np.empty((D, CW + 2 * RPC), np.float16)
        xin[:, 0:CW] = cst
        base = c * RPC
        off = CW
        lo = base
        for N in NS:
            xin[:, off:off + N] = vT[:, lo:lo + N]
            xin[:, off + N:off + 2 * N] = eT[:, lo:lo + N]
            off += 2 * N
            lo += N
        in_maps.append({"xin": xin})
    return in_maps


def _run(in_maps, has_bias=False, trace=False):
    return run_bass_kernel_spmd(_get_nc(has_bias), in_maps,
                                list(range(NCORES)), trace=trace)


def kernel(item_embedding, entity_embedding, w_vv, w_ve, w_ev, w_ee,
           bias_v, bias_e, _trace=False, _res_out=None):
    v = np.asarray(item_embedding, np.float32).reshape(B, D)
    e = np.asarray(entity_embedding, np.float32).reshape(B, D)
    bv = np.asarray(bias_v, np.float32)
    be = np.asarray(bias_e, np.float32)
    has_bias = bool(np.any(bv) or np.any(be))
    in_maps = _make_in_maps(
        v, e,
        np.asarray(w_vv, np.float32), np.asarray(w_ve, np.float32),
        np.asarray(w_ev, np.float32), np.asarray(w_ee, np.float32),
        bv, be)
    res = _run(in_maps, has_bias=has_bias, trace=_trace)
    if _res_out is not None:
        _res_out.append(res)
    item = np.empty((B, D, 1), np.float32)
    ent = np.empty((B, D, 1), np.float32)
    for c in range(NCORES):
        o = res.results[c]["out"]            # [D, 2*RPC] fp16
        base = c * RPC
        off = 0
        lo = base
        for N in NS:
            item[lo:lo + N, :, 0] = o[:, off:off + N].T
            ent[lo:lo + N, :, 0] = o[:, off + N:off + 2 * N].T
            off += 2 * N
            lo += N
    return (item, ent)
